# revision 21
# baseline (speedup 1.0000x reference)
"""2-layer GraphSAGE GNN (segment_sum message passing) on 8 Trainium2 NeuronCores.

Strategy (v2, bf16 pair-gather):
  - dst nodes block-partitioned across 8 cores; incident edges partitioned
    by dst. Within a core, dsts are bin-packed into tiles of <=64 nodes with
    per-half-table edge capacity 512 -> static SPMD schedule on all cores.
  - Edge-source rows fetched with SWDGE dma_gather as bf16 PAIR rows
    (256B = nodes 2u,2u+1 concatenated); table halved so pair indices fit
    int16. 2 gather calls per group per layer (4 SWDGE queues round-robin).
  - Aggregation via PE one-hot matmuls in bf16: a single is_equal builds a
    [slots,128] one-hot whose columns encode (dst_pos + 64*src_parity); two
    accumulating matmuls per 128-slot chunk use the contiguous member slices
    gt[:,0:64]/gt[:,64:128] with oh[:,0:64]/oh[:,64:128].
  - Layer 2 aggregates p = h @ W2_l (transform-first); p exchanged bf16 via
    AllGather; b2 folded into the self-path matmul via an ones-row in hT.
  - log_softmax epilogue on 64-partition tiles; host reassembles rows.
"""
import os
import sys
import types
import contextlib
import ctypes

import numpy as np

# ---------------------------------------------------------------- axon shim
_SO_PATH = "/opt/axon/libaxon_pjrt.so"


def _install_axon_hooks_shim():
    if "antenv.axon_hooks" in sys.modules:
        return
    try:
        lib = ctypes.CDLL(_SO_PATH)
        has = hasattr(lib, "axon_start_nrt_profile")
    except OSError:
        has = False
    if has:
        lib.axon_start_nrt_profile.argtypes = [ctypes.POINTER(ctypes.c_int64), ctypes.c_size_t]
        lib.axon_start_nrt_profile.restype = ctypes.c_int64
        lib.axon_stop_nrt_profile.argtypes = [ctypes.c_char_p]
        lib.axon_stop_nrt_profile.restype = ctypes.c_int64

        @contextlib.contextmanager
        def _hook(output_dir, device_ids):
            import jax

            jax.devices()
            if device_ids:
                ids = (ctypes.c_int64 * len(device_ids))(*device_ids)
                rc = lib.axon_start_nrt_profile(ids, len(device_ids))
            else:
                rc = lib.axon_start_nrt_profile(None, 0)
            if rc != 0:
                raise RuntimeError(f"axon_start_nrt_profile rc={rc}")
            try:
                yield
            finally:
                n = lib.axon_stop_nrt_profile(str(output_dir).encode())
                print(f"ntff profile: {n} file(s) written to {output_dir}", file=sys.stderr)

        hook = _hook
    else:
        hook = None
    mod = types.ModuleType("antenv.axon_hooks")
    mod.get_axon_ntff_profile_hook = lambda: hook
    mod.set_axon_ntff_profile_hook = lambda h: None
    sys.modules["antenv.axon_hooks"] = mod


_install_axon_hooks_shim()

import concourse.bass as bass  # noqa: E402
import concourse.mybir as mybir  # noqa: E402
import concourse.tile as tile  # noqa: E402
import concourse.bacc as bacc  # noqa: E402

NCORES = 8
TW = 64              # dsts per tile
CSLOT = 512          # edge slots per (tile, half)
CPT = CSLOT // 128   # chunks per (tile, half)
NH = 2               # table halves (pair idx must fit int16)
GROUP = int(os.environ.get("KERNEL_GROUP", "2"))  # tiles per group
NQUEUES = 4
F32 = mybir.dt.float32
BF16 = mybir.dt.bfloat16
I16 = mybir.dt.int16
HP = 64              # padded feature width (pair row = 128 bf16 = 256B)


# ------------------------------------------------------------ host: packing
def _pack_core(deg_h, nt):
    """First-fit-decreasing pack of dsts into nt tiles with per-half slot
    capacity CSLOT and <=TW dsts per tile. deg_h: [B, NH] int64.
    Returns (tile_of[B], pos_of[B]) or None."""
    B = deg_h.shape[0]
    order = np.argsort(-deg_h.sum(1), kind="stable")
    cap = np.full((nt, NH), CSLOT, np.int64)
    cnt = np.zeros(nt, np.int64)
    tile_of = np.full(B, -1, np.int64)
    pos_of = np.full(B, -1, np.int64)
    for d in order:
        v = deg_h[d]
        ok = (cap >= v).all(axis=1) & (cnt < TW)
        t = int(np.argmax(ok))
        if not ok[t]:
            return None
        cap[t] -= v
        tile_of[d] = t
        pos_of[d] = cnt[t]
        cnt[t] += 1
    return tile_of, pos_of


# NOTE: L1 buckets edges by src-half h1 = src//HS; L2 needs buckets by
# h2 = row2//HS2.  These differ per edge, so L2 gets its own bucketing pass
# (same tile/pos packing of dsts,独立 slot assignment). Packing capacity
# must hold for BOTH bucketings; _prepare_full below re-packs with the max.
def _prepare_full(x, src, dst):
    N = x.shape[0]
    E = src.shape[0]
    BLOCK = N // NCORES
    HS = N // NH
    src = np.asarray(src, np.int64)
    dst = np.asarray(dst, np.int64)
    core_of_dst = dst // BLOCK
    h1 = src // HS

    deg1 = np.bincount(dst * NH + h1, minlength=N * NH).reshape(N, NH)

    # First pass to get tile/pos/NTC using h1-degrees only is not enough:
    # h2 depends on row_all which depends on packing.  But row_all//HS2 =
    # (core*NTC + tile*TW + pos) // (4*NTC): half2 of a source is simply
    # core//4 (rows of cores 0-3 are half 0, cores 4-7 half 1).  So
    # h2 = src // (N//2) == h1!  (HS2*2 == 8*NTC rows == all cores; half2
    # boundary = core 4 boundary = node N/2.)  Same half split -> one pack.
    per_core_h = np.zeros((NCORES, NH), np.int64)
    for c in range(NCORES):
        per_core_h[c] = deg1[c * BLOCK:(c + 1) * BLOCK].sum(0)
    nt_min = max(int(np.ceil(per_core_h.max() / CSLOT)),
                 int(np.ceil(BLOCK / TW)))
    nt = nt_min
    while True:
        packs = []
        ok = True
        for c in range(NCORES):
            r = _pack_core(deg1[c * BLOCK:(c + 1) * BLOCK], nt)
            if r is None:
                ok = False
                break
            packs.append(r)
        if ok:
            break
        nt += 1
        assert nt <= 254, "packing failed up to NT=254"
    if nt % 2 == 1:
        nt += 1

    NTC = nt * TW
    HS2 = 4 * NTC
    assert 2 * NTC <= 32767

    tile_all = np.empty(N, np.int64)
    pos_all = np.empty(N, np.int64)
    for c in range(NCORES):
        t_of, p_of = packs[c]
        tile_all[c * BLOCK:(c + 1) * BLOCK] = t_of
        pos_all[c * BLOCK:(c + 1) * BLOCK] = p_of
    row_all = (np.arange(N) // BLOCK) * NTC + tile_all * TW + pos_all
    # verify: L2 half of src == L1 half of src
    assert (row_all // HS2 == np.arange(N) // HS).all()

    ngroups = (nt + GROUP - 1) // GROUP
    gsz = [min(GROUP, nt - g * GROUP) for g in range(ngroups)]
    gsz_arr = np.asarray(gsz, np.int64)
    S = nt * NH * CSLOT
    gbase = np.cumsum([0] + [gs * NH * CSLOT for gs in gsz])

    def wrap16(a):
        w = a.reshape(-1, 16).T
        return np.tile(w, (8, 1)).copy()

    import ml_dtypes

    cores = []
    for c in range(NCORES):
        sel = core_of_dst == c
        s_c = src[sel]
        d_c = dst[sel] - c * BLOCK
        h_c = h1[sel]
        t_c = tile_all[c * BLOCK + d_c]
        order = np.lexsort((h_c, t_c))
        s_s, h_s, t_s = s_c[order], h_c[order], t_c[order]
        d_s = d_c[order]
        th = t_s * NH + h_s
        change = np.empty(len(th), bool)
        change[0] = True
        change[1:] = th[1:] != th[:-1]
        run_id = np.cumsum(change) - 1
        starts = np.flatnonzero(change)
        k = np.arange(len(th)) - starts[run_id]
        assert k.max(initial=0) < CSLOT

        g_of_t = t_s // GROUP
        ti = t_s % GROUP
        pos_i = gbase[g_of_t] + h_s * (gsz_arr[g_of_t] * CSLOT) + ti * CSLOT + k
        pos_d = t_s * (NH * CSLOT) + h_s * CSLOT + k

        idx1 = np.zeros(S, np.int16)
        idx1[pos_i] = ((s_s - h_s * HS) // 2).astype(np.int16)
        dl1 = np.full(S, -1.0, np.float32)
        dl1[pos_d] = (pos_all[c * BLOCK + d_s] + TW * (s_s % 2)).astype(np.float32)

        r2 = row_all[s_s]
        idx2 = np.zeros(S, np.int16)
        idx2[pos_i] = ((r2 - h_s * HS2) // 2).astype(np.int16)
        dl2 = np.full(S, -1.0, np.float32)
        dl2[pos_d] = (pos_all[c * BLOCK + d_s] + TW * (r2 % 2)).astype(np.float32)

        def wrap128_bf(a):
            return a.reshape(-1, 128).T.astype(ml_dtypes.bfloat16).copy()

        xT = np.zeros((x.shape[1], NTC), np.float32)
        nodes = np.arange(c * BLOCK, (c + 1) * BLOCK)
        cols = tile_all[nodes] * TW + pos_all[nodes]
        xT[:, cols] = x[nodes].T

        cores.append(dict(idx1=wrap16(idx1), idx2=wrap16(idx2),
                          dl1=wrap128_bf(dl1), dl2=wrap128_bf(dl2),
                          xT=xT.astype(ml_dtypes.bfloat16)))

    meta = dict(N=N, E=E, BLOCK=BLOCK, HS=HS, HS2=HS2, NT=nt, NTC=NTC,
                ngroups=ngroups, gsz=gsz, gbase=gbase, S=S,
                tile_all=tile_all, pos_all=pos_all)
    return cores, meta


# ------------------------------------------------------------ device program
def build_program(meta, IN_FEAT, HIDDEN, NCLS):
    NT, NTC, HS, HS2 = meta["NT"], meta["NTC"], meta["HS"], meta["HS2"]
    ngroups, gsz, gbase, S = meta["ngroups"], meta["gsz"], meta["gbase"], meta["S"]
    N = meta["N"]
    assert IN_FEAT <= HP and HIDDEN == HP and NCLS <= HP

    nc = bacc.Bacc("TRN2", target_bir_lowering=False, num_devices=NCORES,
                   num_swdge_queues=NQUEUES,
                   dynamic_dma_scratch_size=int(os.environ.get("KERNEL_SCRATCH", "98304")))

    t_xp = nc.dram_tensor("xp", [N // 2, HP], F32, kind="ExternalInput")
    t_xT = nc.dram_tensor("xT", [IN_FEAT, NTC], BF16, kind="ExternalInput")
    t_idx1 = nc.dram_tensor("idx1", [128, S // 16], I16, kind="ExternalInput")
    t_idx2 = nc.dram_tensor("idx2", [128, S // 16], I16, kind="ExternalInput")
    t_dl1 = nc.dram_tensor("dl1", [128, S // 128], BF16, kind="ExternalInput")
    t_dl2 = nc.dram_tensor("dl2", [128, S // 128], BF16, kind="ExternalInput")
    t_w1l = nc.dram_tensor("w1l", [HP, HIDDEN], BF16, kind="ExternalInput")
    t_w1r = nc.dram_tensor("w1r", [IN_FEAT, HIDDEN], BF16, kind="ExternalInput")
    t_w2l = nc.dram_tensor("w2l", [HIDDEN, HP], BF16, kind="ExternalInput")
    t_w2r = nc.dram_tensor("w2r65", [HIDDEN + 1, NCLS], BF16, kind="ExternalInput")
    t_b1 = nc.dram_tensor("b1c", [HIDDEN, 1], F32, kind="ExternalInput")
    t_iota = nc.dram_tensor("iota", [128, 128], BF16, kind="ExternalInput")
    t_out = nc.dram_tensor("out", [NTC, NCLS], F32, kind="ExternalOutput")

    AluOp = mybir.AluOpType
    Act = mybir.ActivationFunctionType
    GB = int(os.environ.get("KERNEL_GBUF", "12"))

    with tile.TileContext(nc) as tc:
        with (
            tc.tile_pool(name="const", bufs=1) as constp,
            tc.tile_pool(name="ht", bufs=1) as htp,
            tc.tile_pool(name="gbuf", bufs=GB) as gp,
            tc.tile_pool(name="idxp", bufs=4) as idxp,
            tc.tile_pool(name="ohp", bufs=int(os.environ.get("KERNEL_OHB", "4"))) as ohp,
            tc.tile_pool(name="sb1", bufs=2) as sb1p,
            tc.tile_pool(name="sb2", bufs=2) as sb2p,
            tc.tile_pool(name="dram", bufs=1, space="DRAM") as dramp,
        ):
            iota_sb = constp.tile([128, 128], BF16)
            nc.sync.dma_start(iota_sb[:], t_iota.ap())
            w1l_sb = constp.tile([HP, HIDDEN], BF16)
            nc.sync.dma_start(w1l_sb[:], t_w1l.ap())
            w1r_sb = constp.tile([IN_FEAT, HIDDEN], BF16)
            nc.sync.dma_start(w1r_sb[:], t_w1r.ap())
            w2l_sb = constp.tile([HIDDEN, HP], BF16)
            nc.sync.dma_start(w2l_sb[:], t_w2l.ap())
            w2r_sb = constp.tile([HIDDEN + 1, NCLS], BF16)
            nc.sync.dma_start(w2r_sb[:], t_w2r.ap())
            b1_sb = constp.tile([HIDDEN, 1], F32)
            nc.sync.dma_start(b1_sb[:], t_b1.ap())

            hT = htp.tile([HIDDEN + 1, NTC], BF16)
            nc.vector.memset(hT[HIDDEN:HIDDEN + 1, :], 1.0)
            p_shard = dramp.tile([NTC, HP], BF16)
            p_full = dramp.tile([NCORES * NTC, HP], BF16)

            def load_group_meta(g, t_idx, t_dl):
                Gs = gsz[g]
                base = int(gbase[g])
                callsz = Gs * CSLOT
                idx_g = idxp.tile([128, GROUP * NH * CSLOT // 16], I16,
                                  tag="idxg", name="idxg")
                nc.sync.dma_start(idx_g[:, :NH * callsz // 16],
                                  t_idx.ap()[:, base // 16:(base + NH * callsz) // 16])
                dl_g = idxp.tile([128, GROUP * NH * CPT], BF16, tag="dlg", name="dlg")
                nc.sync.dma_start(dl_g[:, :NH * callsz // 128],
                                  t_dl.ap()[:, base // 128:(base + NH * callsz) // 128])
                return idx_g, dl_g

            def gather_group(g, li, idx_g, table_ap, hsize_pairs):
                """2 gather calls (one per half); table_ap is the pair view
                [rows/2, 128]; hsize_pairs = pair rows per half."""
                Gs = gsz[g]
                callsz = Gs * CSLOT
                gts = []
                for h in range(NH):
                    gt = gp.tile([128, GROUP * CPT, HP], F32, tag="gt", name="gt")
                    nc.gpsimd.dma_gather(
                        gt[:, :Gs * CPT, :],
                        table_ap[h * hsize_pairs:(h + 1) * hsize_pairs, :],
                        idx_g[:, h * callsz // 16:(h + 1) * callsz // 16],
                        callsz, callsz, HP, elem_step=HP,
                        single_packet=False,
                        queue_num=(g * NH + li * ngroups * NH + h) % NQUEUES)
                    gts.append(gt)
                return gts

            def build_oh(dl_g, ti0, ntile):
                """One-hot tile covering tiles [ti0, ti0+ntile) of the group:
                oh[:, tloc*NH*CPT + h*CPT + cc, :]."""
                ncol = ntile * NH * CPT
                oh = ohp.tile([128, 2 * NH * CPT, 128], BF16, tag="oh", name="oh")
                in0 = dl_g[:, ti0 * NH * CPT: ti0 * NH * CPT + ncol]
                in0 = in0.unsqueeze(2).to_broadcast([128, ncol, 128])
                in1 = iota_sb[:].unsqueeze(1).to_broadcast([128, ncol, 128])
                nc.vector.tensor_tensor(oh[:, :ncol, :], in0, in1, op=AluOp.is_equal)
                return oh

            # ---------------- Layer 1 ----------------
            with (
                tc.tile_pool(name="psA", bufs=4, space="PSUM") as psAp,
                tc.tile_pool(name="psz", bufs=2, space="PSUM") as pszp,
                tc.tile_pool(name="psp", bufs=2, space="PSUM") as pspp,
            ):
                xp_pair = t_xp.ap()
                for g in range(ngroups):
                    Gs = gsz[g]
                    idx_g, dl_g = load_group_meta(g, t_idx1, t_dl1)
                    gts = gather_group(g, 0, idx_g, xp_pair, HS // 2)
                    a1sb = sb1p.tile([HP, GROUP * TW], BF16, tag="a1sb", name="a1sb")
                    oh = None
                    for ti in range(Gs):
                        if ti % 2 == 0:
                            oh = build_oh(dl_g, ti, min(2, Gs - ti))
                        psa = psAp.tile([HP, TW], F32, tag="psa", name="psa")
                        nmm = NH * CPT * 2
                        i = 0
                        for h in range(NH):
                            for cc in range(CPT):
                                j = ti * CPT + cc
                                oj = (ti % 2) * NH * CPT + h * CPT + cc
                                for w in range(2):
                                    nc.tensor.matmul(
                                        psa[:],
                                        lhsT=gts[h][:, j, w * 32:(w + 1) * 32].bitcast(BF16),
                                        rhs=oh[:, oj, w * TW:(w + 1) * TW],
                                        start=(i == 0), stop=(i == nmm - 1))
                                    i += 1
                        nc.scalar.copy(a1sb[:, ti * TW:(ti + 1) * TW], psa[:])
                    xT_g = sb1p.tile([IN_FEAT, GROUP * TW], BF16, tag="xtg", name="xtg")
                    nc.sync.dma_start(
                        xT_g[:, :Gs * TW],
                        t_xT.ap()[:, g * GROUP * TW: g * GROUP * TW + Gs * TW])
                    for sub in range((Gs + 7) // 8):
                        w = min(8, Gs - sub * 8) * TW
                        c0 = sub * 512
                        z1 = pszp.tile([HIDDEN, 512], F32, tag="z1", name="z1")
                        nc.tensor.matmul(z1[:, :w], lhsT=w1l_sb[:],
                                         rhs=a1sb[:, c0:c0 + w], start=True, stop=False)
                        nc.tensor.matmul(z1[:, :w], lhsT=w1r_sb[:],
                                         rhs=xT_g[:, c0:c0 + w], start=False, stop=True)
                        h0 = g * GROUP * TW + c0
                        nc.scalar.activation(hT[:HIDDEN, h0:h0 + w], z1[:, :w],
                                             Act.Relu, bias=b1_sb[:, :1])
                        for k in range(w // 128):
                            cc0 = h0 + k * 128
                            pp = pspp.tile([128, HP], F32, tag="pp", name="pp")
                            nc.tensor.matmul(pp[:],
                                             lhsT=hT[:HIDDEN, cc0:cc0 + 128],
                                             rhs=w2l_sb[:], start=True, stop=True)
                            psb = sb2p.tile([128, HP], BF16, tag="psb", name="psb")
                            nc.scalar.copy(psb[:], pp[:])
                            nc.sync.dma_start(p_shard[cc0:cc0 + 128, :], psb[:])

            nc.gpsimd.collective_compute(
                "AllGather", AluOp.bypass,
                replica_groups=[list(range(NCORES))],
                ins=[p_shard.opt()],
                outs=[p_full.opt()],
            )

            # ---------------- Layer 2 ----------------
            with (
                tc.tile_pool(name="pso", bufs=8, space="PSUM") as psop,
                tc.tile_pool(name="smp", bufs=2) as smp,
            ):
                pf_pair = p_full[:, :].bitcast(F32).rearrange("(u w) c -> u (w c)", w=2)
                EPG = int(os.environ.get("KERNEL_EPG", "4"))  # groups per epilogue batch
                o_sb = None
                sg0 = 0
                sgn = 0
                for g in range(ngroups):
                    Gs = gsz[g]
                    idx_g, dl_g = load_group_meta(g, t_idx2, t_dl2)
                    gts = gather_group(g, 1, idx_g, pf_pair, HS2 // 2)
                    if o_sb is None:
                        o_sb = sb2p.tile([TW, EPG * GROUP, NCLS], F32, tag="osb",
                                         name="osb")
                        sg0 = g
                        sgn = 0
                    oh = None
                    for ti in range(Gs):
                        if ti % 2 == 0:
                            oh = build_oh(dl_g, ti, min(2, Gs - ti))
                        po = psop.tile([TW, HP], F32, tag="po", name="po")
                        i = 0
                        for h in range(NH):
                            for cc in range(CPT):
                                j = ti * CPT + cc
                                oj = (ti % 2) * NH * CPT + h * CPT + cc
                                for w in range(2):
                                    nc.tensor.matmul(
                                        po[:],
                                        lhsT=oh[:, oj, w * TW:(w + 1) * TW],
                                        rhs=gts[h][:, j, w * 32:(w + 1) * 32].bitcast(BF16),
                                        start=(i == 0), stop=False)
                                    i += 1
                        tg = g * GROUP + ti
                        nc.tensor.matmul(po[:, :NCLS],
                                         lhsT=hT[:, tg * TW:(tg + 1) * TW],
                                         rhs=w2r_sb[:], start=False, stop=True)
                        nc.scalar.copy(o_sb[:, sgn + ti, :], po[:, :NCLS])
                    sgn += Gs
                    if g % EPG != EPG - 1 and g != ngroups - 1:
                        continue
                    # log_softmax over classes for the supergroup (64 partitions)
                    mx = smp.tile([TW, EPG * GROUP], F32, tag="mx", name="mx")
                    nc.vector.tensor_reduce(mx[:, :sgn], o_sb[:, :sgn, :],
                                            axis=mybir.AxisListType.X, op=AluOp.max)
                    tmp = smp.tile([TW, EPG * GROUP, NCLS], F32, tag="tmp", name="tmp")
                    nc.vector.tensor_tensor(
                        tmp[:, :sgn, :], o_sb[:, :sgn, :],
                        mx[:, :sgn].unsqueeze(2).to_broadcast([TW, sgn, NCLS]),
                        op=AluOp.subtract)
                    ex = smp.tile([TW, EPG * GROUP, NCLS], F32, tag="ex", name="ex")
                    nc.scalar.activation(ex[:, :sgn, :], tmp[:, :sgn, :], Act.Exp)
                    sm = smp.tile([TW, EPG * GROUP], F32, tag="sm", name="sm")
                    nc.vector.tensor_reduce(sm[:, :sgn], ex[:, :sgn, :],
                                            axis=mybir.AxisListType.X, op=AluOp.add)
                    ls = smp.tile([TW, EPG * GROUP], F32, tag="ls", name="ls")
                    nc.scalar.activation(ls[:, :sgn], sm[:, :sgn], Act.Ln)
                    ov = smp.tile([TW, EPG * GROUP, NCLS], F32, tag="ov", name="ov")
                    nc.vector.tensor_tensor(
                        ov[:, :sgn, :], tmp[:, :sgn, :],
                        ls[:, :sgn].unsqueeze(2).to_broadcast([TW, sgn, NCLS]),
                        op=AluOp.subtract)
                    rows0 = sg0 * GROUP * TW
                    out_ap = t_out.ap()[rows0:rows0 + sgn * TW, :].rearrange(
                        "(t p) c -> p t c", p=TW)
                    nc.sync.dma_start(out_ap, ov[:, :sgn, :])
                    o_sb = None

    nc.compile()
    return nc


# ------------------------------------------------------------ runner (PJRT)
class _Runner:
    def __init__(self, nc, n_cores):
        import jax
        from jax.sharding import Mesh, PartitionSpec
        from jax.experimental.shard_map import shard_map
        from concourse.bass2jax import (_bass_exec_p, install_neuronx_cc_hook,
                                        partition_id_tensor)

        install_neuronx_cc_hook()
        self.n_cores = n_cores
        in_names, out_names, out_avals, zero_outs = [], [], [], []
        partition_name = nc.partition_id_tensor.name if nc.partition_id_tensor else None
        for alloc in nc.m.functions[0].allocations:
            if not isinstance(alloc, mybir.MemoryLocationSet):
                continue
            name = alloc.memorylocations[0].name
            if alloc.kind == "ExternalInput":
                if name != partition_name:
                    in_names.append(name)
            elif alloc.kind == "ExternalOutput":
                shape = tuple(alloc.tensor_shape)
                dtype = mybir.dt.np(alloc.dtype)
                out_names.append(name)
                out_avals.append(jax.core.ShapedArray(shape, dtype))
                zero_outs.append(np.zeros(shape, dtype))
        n_params = len(in_names)
        n_outs = len(out_avals)
        all_in = list(in_names) + list(out_names)
        if partition_name is not None:
            all_in.append(partition_name)
        self.in_names, self.out_names, self.zero_outs = in_names, out_names, zero_outs
        donate = tuple(range(n_params, n_params + n_outs))

        def _body(*args):
            operands = list(args)
            if partition_name is not None:
                operands.append(partition_id_tensor())
            outs = _bass_exec_p.bind(
                *operands,
                out_avals=tuple(out_avals),
                in_names=tuple(all_in),
                out_names=tuple(out_names),
                lowering_input_output_aliases=(),
                sim_require_finite=True,
                sim_require_nnan=True,
                nc=nc,
            )
            return tuple(outs)

        devices = jax.devices()[:n_cores]
        self.mesh = Mesh(np.asarray(devices), ("core",))
        in_specs = (PartitionSpec("core"),) * (n_params + n_outs)
        out_specs = (PartitionSpec("core"),) * n_outs
        self.fn = jax.jit(
            shard_map(_body, mesh=self.mesh, in_specs=in_specs,
                      out_specs=out_specs, check_rep=False),
            donate_argnums=donate, keep_unused=True)

    def run(self, in_maps):
        n = self.n_cores
        concat_in = [
            np.concatenate([np.asarray(in_maps[c][nm]) for c in range(n)], axis=0)
            for nm in self.in_names
        ] + [np.concatenate([z] * n, axis=0) for z in self.zero_outs]
        outs = self.fn(*concat_in)
        results = []
        for c in range(n):
            d = {}
            for i, nm in enumerate(self.out_names):
                full = np.asarray(outs[i])
                per = full.shape[0] // n
                d[nm] = full[c * per:(c + 1) * per]
            results.append(d)
        return results


_CACHE = {}


def _get_program(meta, IN_FEAT, HIDDEN, NCLS, use_sim):
    key = (meta["NT"], meta["N"], IN_FEAT, HIDDEN, NCLS, use_sim)
    if key not in _CACHE:
        nc = build_program(meta, IN_FEAT, HIDDEN, NCLS)
        _CACHE[key] = (nc, None)
    return _CACHE[key]


def _make_in_maps(x, W1_l, W1_r, b1, W2_l, W2_r, b2, cores):
    import ml_dtypes
    N, IN_FEAT = x.shape
    HIDDEN = W1_l.shape[1]
    NCLS = W2_l.shape[1]
    xpad = np.zeros((N, HP), np.float32)
    xpad[:, :IN_FEAT] = x
    xp = xpad.reshape(N // 2, 2 * HP).astype(ml_dtypes.bfloat16).view(np.float32)
    w1l_pad = np.zeros((HP, HIDDEN), np.float32)
    w1l_pad[:IN_FEAT] = W1_l
    w2l_pad = np.zeros((HIDDEN, HP), np.float32)
    w2l_pad[:, :NCLS] = W2_l
    w2r65 = np.concatenate([W2_r, b2.reshape(1, NCLS)], axis=0)
    iota = np.broadcast_to(np.arange(128, dtype=np.float32), (128, 128))

    bf = ml_dtypes.bfloat16
    in_maps = []
    for c in range(NCORES):
        in_maps.append({
            "xp": xp,
            "xT": cores[c]["xT"],
            "idx1": cores[c]["idx1"],
            "idx2": cores[c]["idx2"],
            "dl1": cores[c]["dl1"],
            "dl2": cores[c]["dl2"],
            "w1l": w1l_pad.astype(bf),
            "w1r": W1_r.astype(bf),
            "w2l": w2l_pad.astype(bf),
            "w2r65": w2r65.astype(bf),
            "b1c": b1.reshape(HIDDEN, 1),
            "iota": iota.astype(bf),
        })
    return in_maps


def kernel(x, W1_l, W1_r, b1, W2_l, W2_r, b2, src, dst):
    x = np.asarray(x, np.float32)
    W1_l = np.asarray(W1_l, np.float32)
    W1_r = np.asarray(W1_r, np.float32)
    b1 = np.asarray(b1, np.float32)
    W2_l = np.asarray(W2_l, np.float32)
    W2_r = np.asarray(W2_r, np.float32)
    b2 = np.asarray(b2, np.float32)

    N, IN_FEAT = x.shape
    HIDDEN = W1_l.shape[1]
    NCLS = W2_l.shape[1]

    cores, meta = _prepare_full(x, src, dst)

    use_sim = os.environ.get("KERNEL_SIM", "0") == "1"
    nc, runner = _get_program(meta, IN_FEAT, HIDDEN, NCLS, use_sim)

    in_maps = _make_in_maps(x, W1_l, W1_r, b1, W2_l, W2_r, b2, cores)

    if use_sim:
        from concourse.bass_interp import MultiCoreSim
        sim = MultiCoreSim(nc, num_cores=NCORES, trace=False)
        for c, core in sim.cores.items():
            core.assign_tensors(in_maps[c])
        sim.simulate(check_with_hw=False)
        results = [{"out": sim.cores[c].tensor("out").copy()} for c in range(NCORES)]
    else:
        from concourse import bass2jax
        results = bass2jax.run_bass_via_pjrt(nc, in_maps, n_cores=NCORES)

    out = np.empty((N, NCLS), np.float32)
    BLOCK = meta["BLOCK"]
    rows_local = meta["tile_all"] * TW + meta["pos_all"]
    for c in range(NCORES):
        nodes = np.arange(c * BLOCK, (c + 1) * BLOCK)
        out[nodes] = results[c]["out"][rows_local[nodes]]
    return out
